# revision 1
# baseline (speedup 1.0000x reference)
"""Trainium2 Bass kernel for nn_Attn_128849019074 (sparse_attention).

reference:
    energy = einsum("lbd,ed->lbe", enc, W) + b        # [L,B,D] huge matmul
    scores = einsum("lbd,bd->lb", energy, hidden)     # [L,B]
    out    = log_softmax(scores, axis=1)[None, None]  # [1,1,L,B]

Algebraic rewrite (linearity):
    scores[l,b] = enc[l,b,:] . v[b,:] + c[b]
    with v = hidden @ W   ([B,D], tiny)  and  c = hidden @ b  ([B]).

This turns a 137-GMAC matmul into a single streaming pass over
encoder_outputs (268 MB) -> memory bound.

Distribution: shard over L (128 timesteps per core).  The dim=1
log-softmax is over B=32, which is fully local per (l) row -> no
collective needed for the softmax.  v is computed column-sharded
(each core does 256 of the 2048 columns on the PE) and AllGather'd.

Per-core dataflow:
  - enc chunk arrives as 8 contiguous 4-MB DMAs into SBUF tiles
    [128 partitions = (4 l's x 32 b's), 4, 2048].
  - one fused DVE tensor_tensor_reduce per l-group:
        prod = enc_tile * vbig ; scores[:, col] = sum_d prod
    where vbig[p, d] = v[p % 32, d] (v replicated 4x on partitions).
  - scores [128, 32] -> +c -> PE transpose -> [32 tiles, 128=(4a x 32b)]
    -> segmented (per-32) max / exp-accum / ln -> out rows l = 4t + a.
"""

import os
import sys

sys.path.insert(0, "/opt/trn_rl_repo")

import numpy as np

L = 1024
B = 32
D = 2048
NCORES = 8
L_LOC = L // NCORES          # 128 timesteps per core
D_SH = D // NCORES           # 256 v-columns computed per core
G = 4                        # l-groups per enc tile
N_TILES = L_LOC // (4 * G)   # 8 enc DMAs of [128, G, D] (4 MB each)
N_COLS = L_LOC // 4          # 32 score columns (one per l-group)

_CACHE: dict = {}
last_results = None          # BassKernelResults from the most recent run


def _split_drain_waits(nc):
    """Walrus rejects Drain instructions carrying many sync waits ("Too many
    sync wait commands").  Tile's kernel-tail drain waits on every live
    semaphore lane at once; split it into a chain of single-wait drains."""
    import concourse.mybir as mybir

    for bb in nc.main_func.blocks:
        idx = 0
        while idx < len(bb.instructions):
            inst = bb.instructions[idx]
            if (
                isinstance(inst, mybir.InstDrain)
                and inst.sync_info is not None
                and len(inst.sync_info.on_wait or []) > 1
            ):
                waits = list(inst.sync_info.on_wait)
                spill, keep = waits[:-1], waits[-1:]
                new_insts = []
                for j, w in enumerate(spill):
                    x = mybir.InstDrain(name=f"{inst.name}_w{j}", ins=[], outs=[])
                    x.engine = inst.engine
                    x.sync_info = mybir.SyncInfo(on_wait=[w], on_update=[])
                    x.debug = inst.debug
                    nc.register_instruction(x)
                    new_insts.append(x)
                inst.sync_info = mybir.SyncInfo(
                    on_wait=keep, on_update=list(inst.sync_info.on_update or [])
                )
                bb.instructions[idx:idx] = new_insts
                idx += len(new_insts)
            idx += 1


def build_program():
    """Build (once) the SPMD Bass program shared by all 8 cores."""
    if "nc" in _CACHE:
        return _CACHE["nc"]

    import concourse.bacc as bacc
    import concourse.mybir as mybir
    import concourse.tile as tile

    f32 = mybir.dt.float32
    Alu = mybir.AluOpType
    Act = mybir.ActivationFunctionType

    nc = bacc.Bacc(
        "TRN2", target_bir_lowering=False, debug=False, num_devices=NCORES
    )

    enc = nc.dram_tensor("enc", [L_LOC * B, D], f32, kind="ExternalInput").ap()
    # hbtt[p, 33c + j] = [hidden; b]^T[128c + p, j] — pre-tiled on the host so
    # the DMA is one contiguous run per partition.
    hbtt = nc.dram_tensor(
        "hbtt", [128, 16 * (B + 1)], f32, kind="ExternalInput"
    ).ap()
    wfull = nc.dram_tensor("wfull", [D, D], f32, kind="ExternalInput").ap()
    out = nc.dram_tensor("out", [L_LOC, B], f32, kind="ExternalOutput").ap()
    ident = nc.inline_tensor(np.eye(128, dtype=np.float32), "ident").ap()
    # repl[b, p] = 1 if p % 32 == b — PE-side partition replication matrix
    repl = nc.inline_tensor(
        np.ascontiguousarray(np.tile(np.eye(B, dtype=np.float32), (1, 4))),
        "repl",
    ).ap()

    with tile.TileContext(nc) as tc:
        with (
            tc.tile_pool(name="pers", bufs=1) as pers,
            tc.tile_pool(name="encp", bufs=3) as encp,
            tc.tile_pool(name="wp", bufs=6) as wp,
            tc.tile_pool(name="prodp", bufs=2) as prodp,
            tc.tile_pool(name="psp", bufs=1, space="PSUM") as psp,
        ):
            # ---------- phase 1: v = hidden @ W and c = hidden @ b on PE ----
            # No collectives: every core reads the full (replicated) W.  The
            # NEFF-start barrier + collective trigger latency (~75 us measured)
            # costs far more than the extra 16 MB W read (~45 us).
            hbt_sb = pers.tile([128, 16 * (B + 1)], f32)
            nc.sync.dma_start(hbt_sb[:, :], hbtt[:, :])
            ident_sb = pers.tile([128, 128], f32)
            nc.sync.dma_start(ident_sb[:, :], ident[:, :])
            repl_sb = pers.tile([B, 128], f32)
            nc.sync.dma_start(repl_sb[:, :], repl[:, :])

            # PE warm-up: ~4.5 us of back-to-back matmuls flips the HAM clock
            # gate from 1.2 to 2.4 GHz before the W-chunk matmuls begin.
            warm_ps = psp.tile([128, 512], f32)
            for i in range(10):
                nc.tensor.matmul(
                    warm_ps[:, :], hbt_sb[:, 0:128], hbt_sb[:, 0:512]
                )
            warm_junk = pers.tile([1, 1], f32)
            nc.vector.tensor_copy(warm_junk[:, :], warm_ps[0:1, 0:1])

            v_ps = psp.tile([B, D], f32, tag="big")
            c_ps = psp.tile([B, 1], f32)
            last_w_dma = None
            for c in range(16):
                wc = wp.tile([128, D], f32, tag="wc")
                last_w_dma = nc.sync.dma_start(
                    wc[:, :], wfull[128 * c : 128 * (c + 1), :]
                )
                lhs = hbt_sb[:, (B + 1) * c : (B + 1) * c + B]
                for n in range(4):
                    nc.tensor.matmul(
                        v_ps[:, 512 * n : 512 * (n + 1)],
                        lhs,
                        wc[:, 512 * n : 512 * (n + 1)],
                        start=(c == 0),
                        stop=(c == 15),
                    )
            for c in range(16):
                lhs = hbt_sb[:, (B + 1) * c : (B + 1) * c + B]
                rhs = hbt_sb[:, (B + 1) * c + B : (B + 1) * (c + 1)]
                nc.tensor.matmul(
                    c_ps[:, :], lhs, rhs, start=(c == 0), stop=(c == 15)
                )

            # ---------- phase 2: replicate v/c 4x across partitions via PE --
            # vbig[32a + b, d] = v[b, d]; staged through rows 0-31 of vbig.
            vbig = pers.tile([128, D], f32)
            nc.vector.tensor_copy(vbig[0:B, :], v_ps[:, :])
            vbig_ps = psp.tile([128, D], f32, tag="big")
            for n in range(4):
                nc.tensor.matmul(
                    vbig_ps[:, 512 * n : 512 * (n + 1)],
                    repl_sb[:, :],
                    vbig[0:B, 512 * n : 512 * (n + 1)],
                )
            nc.vector.tensor_copy(vbig[:, :], vbig_ps[:, :])
            cbig = pers.tile([128, 1], f32)
            nc.vector.tensor_copy(cbig[0:B, :], c_ps[:, :])
            cbig_ps = psp.tile([128, 1], f32, tag="small_ps")
            nc.tensor.matmul(cbig_ps[:, :], repl_sb[:, :], cbig[0:B, :])
            nc.vector.tensor_copy(cbig[:, :], cbig_ps[:, :])

            # ---------- phase 3: stream enc, dot with v ----------
            # DVE does the elementwise multiply; the (otherwise idle) ACT
            # engine does the free-axis reduction via activation accum_out.
            scores = pers.tile([128, N_COLS], f32)
            ascr = pers.tile([128, D], f32)  # ACT dummy out
            for t in range(N_TILES):
                et = encp.tile([128, G, D], f32, tag="et")
                enc_dma = nc.sync.dma_start(
                    et[:, :, :],
                    enc[128 * G * t : 128 * G * (t + 1), :].rearrange(
                        "(g p) d -> p g d", p=128
                    ),
                )
                # Keep the HWDGE FIFO ring W-first: v (and thus vbig) must be
                # ready early so the DVE can drain enc tiles as they land.
                tile.add_dep_helper(
                    enc_dma.ins,
                    last_w_dma.ins,
                    sync=False,
                    reason="enc stream after W (W-first DMA ordering)",
                )
                for g in range(G):
                    pt = prodp.tile([128, D], f32, tag="pt")
                    nc.vector.tensor_mul(pt[:, :], et[:, g, :], vbig[:, :])
                    nc.scalar.activation(
                        ascr[:, :],
                        pt[:, :],
                        Act.Copy,
                        accum_out=scores[:, G * t + g : G * t + g + 1],
                    )

            # ---------- phase 4: + c, transpose, log-softmax over b ----------
            sc2 = pers.tile([128, N_COLS], f32)
            nc.vector.tensor_scalar_add(sc2[:, :], scores[:, :], cbig[:, 0:1])
            scT_ps = psp.tile([N_COLS, 128], f32, tag="small_ps")
            nc.tensor.transpose(scT_ps[:, :], sc2[:, :], ident_sb[:, :])
            tsb = pers.tile([N_COLS, 128], f32)
            nc.vector.tensor_copy(tsb[:, :], scT_ps[:, :])
            m4 = pers.tile([N_COLS, 4], f32)
            nc.vector.tensor_reduce(
                m4[:, :],
                tsb.rearrange("p (a b) -> p a b", b=B),
                axis=mybir.AxisListType.X,
                op=Alu.max,
            )
            sm = pers.tile([N_COLS, 128], f32)
            s4 = pers.tile([N_COLS, 4], f32)
            es = pers.tile([N_COLS, 128], f32)
            for a in range(4):
                nc.vector.tensor_scalar_sub(
                    sm[:, B * a : B * (a + 1)],
                    tsb[:, B * a : B * (a + 1)],
                    m4[:, a : a + 1],
                )
            for a in range(4):
                nc.scalar.activation(
                    es[:, B * a : B * (a + 1)],
                    sm[:, B * a : B * (a + 1)],
                    Act.Exp,
                    accum_out=s4[:, a : a + 1],
                )
            ls4 = pers.tile([N_COLS, 4], f32)
            nc.scalar.activation(ls4[:, :], s4[:, :], Act.Ln)
            o = pers.tile([N_COLS, 128], f32)
            for a in range(4):
                nc.vector.tensor_scalar_sub(
                    o[:, B * a : B * (a + 1)],
                    sm[:, B * a : B * (a + 1)],
                    ls4[:, a : a + 1],
                )
            # out[4t + a, b] = o[t, 32a + b]
            out_r = out.rearrange("(t a) b -> t a b", a=4)
            for a in range(4):
                nc.sync.dma_start(out_r[:, a, :], o[:, B * a : B * (a + 1)])

    nc.compile()
    _split_drain_waits(nc)
    _CACHE["nc"] = nc
    return nc


def make_in_maps(hidden, encoder_outputs, W, b):
    hidden = np.ascontiguousarray(np.asarray(hidden, dtype=np.float32))
    enc = np.ascontiguousarray(np.asarray(encoder_outputs, dtype=np.float32))
    W_ = np.ascontiguousarray(np.asarray(W, dtype=np.float32))
    b_ = np.ascontiguousarray(np.asarray(b, dtype=np.float32))
    hb = np.concatenate([hidden, b_[None, :]], axis=0)  # [33, D]
    # hbtt[p, 33c + j] = hb[j, 128c + p] — the SBUF tile layout, host-built
    hbtt = np.ascontiguousarray(
        hb.T.reshape(16, 128, B + 1).transpose(1, 0, 2).reshape(128, 16 * (B + 1))
    )
    in_maps = []
    for k in range(NCORES):
        in_maps.append(
            {
                "enc": np.ascontiguousarray(
                    enc[k * L_LOC : (k + 1) * L_LOC].reshape(L_LOC * B, D)
                ),
                "hbtt": hbtt,
                "wfull": W_,
            }
        )
    return in_maps


def kernel(hidden, encoder_outputs, W, b):
    """Full inputs in, full [1, 1, L, B] output out; runs on 8 NeuronCores."""
    global last_results
    from concourse.bass_utils import run_bass_kernel_spmd

    nc = build_program()
    in_maps = make_in_maps(hidden, encoder_outputs, W, b)
    res = run_bass_kernel_spmd(
        nc,
        in_maps,
        list(range(NCORES)),
        trace=bool(os.environ.get("KERNEL_TRACE")),
    )
    last_results = res
    chunks = [res.results[k]["out"] for k in range(NCORES)]
    full = np.concatenate(chunks, axis=0).reshape(1, 1, L, B)
    return full.astype(np.float32)



# revision 4
# speedup vs baseline: 1.4147x; 1.4147x over previous
"""Trainium2 Bass kernel for nn_Attn_128849019074 (sparse_attention).

reference:
    energy = einsum("lbd,ed->lbe", enc, W) + b        # [L,B,D] huge matmul
    scores = einsum("lbd,bd->lb", energy, hidden)     # [L,B]
    out    = log_softmax(scores, axis=1)[None, None]  # [1,1,L,B]

Algebraic rewrite (linearity):
    scores[l,b] = enc[l,b,:] . v[b,:] + c[b]
    with v = hidden @ W   ([B,D], tiny)  and  c = hidden @ b  ([B]).

This turns a 137-GMAC matmul into a single streaming pass over
encoder_outputs -> memory bound.  All streamed operands are cast to
bf16 on the host (rel err ~2.8e-3, tolerance 2e-2), halving HBM
traffic (enc 16.8 MB + W 8.4 MB per core) and unlocking the 2x DVE
mode and single-pass (non-LOW_HIGH) PE matmuls.

Distribution: shard over L (128 timesteps per core).  The dim=1
log-softmax is over B=32, which is local per (l) row -> no collective.
W is replicated (a v-AllGather's ~20-75 us trigger latency exceeds the
23 us bf16 W read).

Per-core dataflow:
  - W arrives as 16 chunk DMAs; v = hidden @ W accumulates on the PE
    behind each chunk (bf16, 1024-wide moving operand).
  - v replicated 4x across partitions via PE (repl matrix), then tiled
    4x along the free axis -> vbigG [128, 4*2048] bf16.
  - enc chunk arrives as 8 contiguous 2.1-MB DMAs, host-packed so tile
    t partition (32a+b) holds enc[l=16t+4g+a, b, :] for g=0..3.
  - per tile: ONE DVE tensor_mul [128, 8192] (bf16 2x mode), then per
    l-group the free-axis sum: groups 0-1 on ACT (activation accum_out),
    groups 2-3 on DVE (tensor_reduce, 4x mode) -> scores [128, 32] f32.
  - scores -> +c -> PE transpose -> [32, 128] -> segmented (per-32)
    max / exp-accum / ln -> out rows l = 4*col + a.  All f32.
"""

import os
import sys

sys.path.insert(0, "/opt/trn_rl_repo")

import numpy as np

L = 1024
B = 32
D = 2048
NCORES = 8
L_LOC = L // NCORES          # 128 timesteps per core
G = 4                        # l-groups per enc tile
N_TILES = L_LOC // (4 * G)   # 8 enc DMAs of [128, G*D] bf16 (2.1 MB each)
N_COLS = L_LOC // 4          # 32 score columns (one per l-group)
ACT_GROUPS = (0, 1)          # reduce on ACT; the rest on DVE tensor_reduce

_CACHE: dict = {}
last_results = None          # BassKernelResults from the most recent run


def _split_drain_waits(nc):
    """Walrus rejects Drain instructions carrying many sync waits ("Too many
    sync wait commands").  Tile's kernel-tail drain waits on every live
    semaphore lane at once; split it into a chain of single-wait drains."""
    import concourse.mybir as mybir

    for bb in nc.main_func.blocks:
        idx = 0
        while idx < len(bb.instructions):
            inst = bb.instructions[idx]
            if (
                isinstance(inst, mybir.InstDrain)
                and inst.sync_info is not None
                and len(inst.sync_info.on_wait or []) > 1
            ):
                waits = list(inst.sync_info.on_wait)
                spill, keep = waits[:-1], waits[-1:]
                new_insts = []
                for j, w in enumerate(spill):
                    x = mybir.InstDrain(name=f"{inst.name}_w{j}", ins=[], outs=[])
                    x.engine = inst.engine
                    x.sync_info = mybir.SyncInfo(on_wait=[w], on_update=[])
                    x.debug = inst.debug
                    nc.register_instruction(x)
                    new_insts.append(x)
                inst.sync_info = mybir.SyncInfo(
                    on_wait=keep, on_update=list(inst.sync_info.on_update or [])
                )
                bb.instructions[idx:idx] = new_insts
                idx += len(new_insts)
            idx += 1


def build_program():
    """Build (once) the SPMD Bass program shared by all 8 cores."""
    if "nc" in _CACHE:
        return _CACHE["nc"]

    import ml_dtypes
    import concourse.bacc as bacc
    import concourse.mybir as mybir
    import concourse.tile as tile

    f32 = mybir.dt.float32
    bf16 = mybir.dt.bfloat16
    Alu = mybir.AluOpType
    Act = mybir.ActivationFunctionType

    nc = bacc.Bacc(
        "TRN2", target_bir_lowering=False, debug=False, num_devices=NCORES
    )

    enc = nc.dram_tensor("enc", [N_TILES * 128, G * D], bf16, kind="ExternalInput").ap()
    # hbtt[p, 33c + j] = [hidden; b]^T[128c + p, j] — pre-tiled on the host so
    # the DMA is one contiguous run per partition.
    hbtt = nc.dram_tensor(
        "hbtt", [128, 16 * (B + 1)], bf16, kind="ExternalInput"
    ).ap()
    wfull = nc.dram_tensor("wfull", [D, D], bf16, kind="ExternalInput").ap()
    out = nc.dram_tensor("out", [L_LOC, B], f32, kind="ExternalOutput").ap()
    ident = nc.inline_tensor(np.eye(128, dtype=np.float32), "ident").ap()
    # repl[b, p] = 1 if p % 32 == b — PE-side partition replication matrix
    repl = nc.inline_tensor(
        np.ascontiguousarray(
            np.tile(np.eye(B, dtype=np.float32), (1, 4)).astype(ml_dtypes.bfloat16)
        ),
        "repl",
    ).ap()

    with tile.TileContext(nc) as tc:
        with (
            tc.tile_pool(name="pers", bufs=1) as pers,
            tc.tile_pool(name="encp", bufs=3) as encp,
            tc.tile_pool(name="wp", bufs=6) as wp,
            tc.tile_pool(name="prodp", bufs=2) as prodp,
            tc.tile_pool(name="psp", bufs=1, space="PSUM") as psp,
        ):
            # ---------- phase 1: v = hidden @ W and c = hidden @ b on PE ----
            hbt_sb = pers.tile([128, 16 * (B + 1)], bf16)
            nc.sync.dma_start(hbt_sb[:, :], hbtt[:, :])
            repl_sb = pers.tile([B, 128], bf16)
            nc.sync.dma_start(repl_sb[:, :], repl[:, :])

            # PE warm-up: ~4 us of back-to-back matmuls flips the HAM clock
            # gate from 1.2 to 2.4 GHz before the W-chunk matmuls begin.
            warm_ps = psp.tile([128, 512], f32)
            for i in range(12):
                nc.tensor.matmul(
                    warm_ps[:, :], hbt_sb[:, 0:128], hbt_sb[:, 0:512]
                )
            warm_junk = pers.tile([1, 1], f32)
            nc.vector.tensor_copy(warm_junk[:, :], warm_ps[0:1, 0:1])

            v_ps = psp.tile([B, D], f32, tag="big")
            c_ps = psp.tile([B, 1], f32)
            last_w_dma = None
            for c in range(16):
                wc = wp.tile([128, D], bf16, tag="wc")
                last_w_dma = nc.sync.dma_start(
                    wc[:, :], wfull[128 * c : 128 * (c + 1), :]
                )
                lhs = hbt_sb[:, (B + 1) * c : (B + 1) * c + B]
                for n in range(4):
                    nc.tensor.matmul(
                        v_ps[:, 512 * n : 512 * (n + 1)],
                        lhs,
                        wc[:, 512 * n : 512 * (n + 1)],
                        start=(c == 0),
                        stop=(c == 15),
                    )
            for c in range(16):
                lhs = hbt_sb[:, (B + 1) * c : (B + 1) * c + B]
                rhs = hbt_sb[:, (B + 1) * c + B : (B + 1) * (c + 1)]
                nc.tensor.matmul(
                    c_ps[:, :], lhs, rhs, start=(c == 0), stop=(c == 15)
                )

            # ---------- phase 2: replicate v/c 4x across partitions via PE --
            # vbigG[32a + b, 2048 g + d] = v[b, d] (4x partitions, 4x free)
            vtmp = pers.tile([B, D], bf16)
            nc.vector.tensor_copy(vtmp[:, :], v_ps[:, :])
            vbig_ps = psp.tile([128, D], f32, tag="big")
            for n in range(4):
                nc.tensor.matmul(
                    vbig_ps[:, 512 * n : 512 * (n + 1)],
                    repl_sb[:, :],
                    vtmp[:, 512 * n : 512 * (n + 1)],
                )
            vbigG = pers.tile([128, G * D], bf16)
            nc.vector.tensor_copy(vbigG[:, 0:D], vbig_ps[:, :])
            for g in range(1, G):
                nc.vector.tensor_copy(vbigG[:, g * D : (g + 1) * D], vbigG[:, 0:D])
            ctmp = pers.tile([B, 1], bf16)
            nc.vector.tensor_copy(ctmp[:, :], c_ps[:, :])
            cbig_ps = psp.tile([128, 1], f32, tag="small_ps")
            nc.tensor.matmul(cbig_ps[:, :], repl_sb[:, :], ctmp[:, :])
            cbig = pers.tile([128, 1], f32)
            nc.vector.tensor_copy(cbig[:, :], cbig_ps[:, :])

            # ---------- phase 3: stream enc, dot with v ----------
            # One whole-tile DVE multiply (bf16 2x mode), then the free-axis
            # reduction split across ACT (accum_out) and DVE (tensor_reduce,
            # 4x mode) so neither trails the DMA stream.
            scores = pers.tile([128, N_COLS], f32)
            ascr = pers.tile([128, D], bf16)  # ACT dummy out
            for t in range(N_TILES):
                et = encp.tile([128, G * D], bf16, tag="et")
                enc_dma = nc.sync.dma_start(
                    et[:, :], enc[128 * t : 128 * (t + 1), :]
                )
                # Keep the HWDGE FIFO ring W-first: v (and thus vbigG) must be
                # ready early so the DVE can drain enc tiles as they land.
                tile.add_dep_helper(
                    enc_dma.ins,
                    last_w_dma.ins,
                    sync=False,
                    reason="enc stream after W (W-first DMA ordering)",
                )
                pt = prodp.tile([128, G * D], bf16, tag="pt")
                nc.vector.tensor_mul(pt[:, :], et[:, :], vbigG[:, :])
                for g in range(G):
                    col = G * t + g
                    if g in ACT_GROUPS:
                        nc.scalar.activation(
                            ascr[:, :],
                            pt[:, g * D : (g + 1) * D],
                            Act.Copy,
                            accum_out=scores[:, col : col + 1],
                        )
                    else:
                        nc.vector.tensor_reduce(
                            scores[:, col : col + 1],
                            pt[:, g * D : (g + 1) * D],
                            axis=mybir.AxisListType.X,
                            op=Alu.add,
                        )

            # ident only needed by the epilogue transpose; keep its DMA
            # behind the enc stream in the queue.
            ident_sb = pers.tile([128, 128], f32)
            ident_dma = nc.sync.dma_start(ident_sb[:, :], ident[:, :])
            tile.add_dep_helper(
                ident_dma.ins,
                last_w_dma.ins,
                sync=False,
                reason="ident after W",
            )

            # ---------- phase 4: + c, transpose, log-softmax over b ----------
            sc2 = pers.tile([128, N_COLS], f32)
            nc.vector.tensor_scalar_add(sc2[:, :], scores[:, :], cbig[:, 0:1])
            scT_ps = psp.tile([N_COLS, 128], f32, tag="small_ps")
            nc.tensor.transpose(scT_ps[:, :], sc2[:, :], ident_sb[:, :])
            tsb = pers.tile([N_COLS, 128], f32)
            nc.vector.tensor_copy(tsb[:, :], scT_ps[:, :])
            m4 = pers.tile([N_COLS, 4], f32)
            nc.vector.tensor_reduce(
                m4[:, :],
                tsb.rearrange("p (a b) -> p a b", b=B),
                axis=mybir.AxisListType.X,
                op=Alu.max,
            )
            sm = pers.tile([N_COLS, 128], f32)
            s4 = pers.tile([N_COLS, 4], f32)
            es = pers.tile([N_COLS, 128], f32)
            for a in range(4):
                nc.vector.tensor_scalar_sub(
                    sm[:, B * a : B * (a + 1)],
                    tsb[:, B * a : B * (a + 1)],
                    m4[:, a : a + 1],
                )
            for a in range(4):
                nc.scalar.activation(
                    es[:, B * a : B * (a + 1)],
                    sm[:, B * a : B * (a + 1)],
                    Act.Exp,
                    accum_out=s4[:, a : a + 1],
                )
            ls4 = pers.tile([N_COLS, 4], f32)
            nc.scalar.activation(ls4[:, :], s4[:, :], Act.Ln)
            o = pers.tile([N_COLS, 128], f32)
            for a in range(4):
                nc.vector.tensor_scalar_sub(
                    o[:, B * a : B * (a + 1)],
                    sm[:, B * a : B * (a + 1)],
                    ls4[:, a : a + 1],
                )
            # out[4t + a, b] = o[t, 32a + b]
            out_r = out.rearrange("(t a) b -> t a b", a=4)
            for a in range(4):
                nc.sync.dma_start(out_r[:, a, :], o[:, B * a : B * (a + 1)])

    nc.compile()
    _split_drain_waits(nc)
    _CACHE["nc"] = nc
    return nc


def make_in_maps(hidden, encoder_outputs, W, b):
    import ml_dtypes

    bf16 = ml_dtypes.bfloat16
    hidden = np.asarray(hidden, dtype=np.float32)
    enc16 = np.asarray(encoder_outputs, dtype=np.float32).astype(bf16)
    W16 = np.ascontiguousarray(np.asarray(W, dtype=np.float32).astype(bf16))
    b_ = np.asarray(b, dtype=np.float32)
    hb = np.concatenate([hidden, b_[None, :]], axis=0)  # [33, D]
    # hbtt[p, 33c + j] = hb[j, 128c + p] — the SBUF tile layout, host-built
    hbtt16 = np.ascontiguousarray(
        hb.T.reshape(16, 128, B + 1)
        .transpose(1, 0, 2)
        .reshape(128, 16 * (B + 1))
        .astype(bf16)
    )
    in_maps = []
    for k in range(NCORES):
        # tile t, partition 32a+b, free g*D+d  <-  enc[l = 16t+4g+a, b, d]
        ek = (
            enc16[k * L_LOC : (k + 1) * L_LOC]
            .reshape(N_TILES, G, 4, B, D)
            .transpose(0, 2, 3, 1, 4)
            .reshape(N_TILES * 128, G * D)
        )
        in_maps.append(
            {
                "enc": np.ascontiguousarray(ek),
                "hbtt": hbtt16,
                "wfull": W16,
            }
        )
    return in_maps


def kernel(hidden, encoder_outputs, W, b):
    """Full inputs in, full [1, 1, L, B] output out; runs on 8 NeuronCores."""
    global last_results
    from concourse.bass_utils import run_bass_kernel_spmd

    nc = build_program()
    in_maps = make_in_maps(hidden, encoder_outputs, W, b)
    res = run_bass_kernel_spmd(
        nc,
        in_maps,
        list(range(NCORES)),
        trace=bool(os.environ.get("KERNEL_TRACE")),
    )
    last_results = res
    chunks = [res.results[k]["out"] for k in range(NCORES)]
    full = np.concatenate(chunks, axis=0).reshape(1, 1, L, B)
    return full.astype(np.float32)


# revision 9
# speedup vs baseline: 1.5565x; 1.1002x over previous
"""Trainium2 Bass kernel for nn_Attn_128849019074 (sparse_attention).

reference:
    energy = einsum("lbd,ed->lbe", enc, W) + b        # [L,B,D] huge matmul
    scores = einsum("lbd,bd->lb", energy, hidden)     # [L,B]
    out    = log_softmax(scores, axis=1)[None, None]  # [1,1,L,B]

Algebraic rewrite (linearity):
    scores[l,b] = enc[l,b,:] . v[b,:] + c[b]
    with v = hidden @ W   ([B,D], tiny)  and  c = hidden @ b  ([B]).

This turns a 137-GMAC matmul into a single streaming pass over
encoder_outputs -> memory bound.  All streamed operands are cast to
bf16 on the host (rel err ~3e-3, tolerance 2e-2), halving HBM traffic
(enc 16.8 MB + W 8.4 MB per core).

Distribution: shard over L (128 timesteps per core).  The dim=1
log-softmax is over B=32, fully local per l -> no collectives.  W is
replicated (a v-AllGather's ~20-75 us trigger latency exceeds the
23 us bf16 W read).

Compute mapping: the multiply-reduce runs on the idle *TensorEngine*
(DVE tensor ops measure ~2.2 us per [128,2048] on HW -- far too slow
to keep up with the DMA stream).  enc is host-transposed so the
contraction dim d sits on partitions:

  - enc tile j [128, 16, 512] bf16: partition p, free (c, 32*lg+b) =
    enc[l = 16j+lg, b, 128c+p].
  - T_ps[b', col] += vT[:, c, b'].T @ et[:, c, :]   (16 chunk matmuls
    per tile, f32 PSUM accumulation = the full d-dot product).
  - diag extract: masked = T_ps * mask (DVE; mask[b', 32lg+b] = b'==b),
    then ones-matmul collapses partitions: sps[j%4, col] = sum_b' masked.
  - epilogue per 4-block half: +c, max, exp, ln on [4, (16, 32)] rows,
    one contiguous out DMA.  First half hidden under the stream.
"""

import os
import sys

sys.path.insert(0, "/opt/trn_rl_repo")

import numpy as np

L = 1024
B = 32
D = 2048
NCORES = 8
L_LOC = L // NCORES          # 128 timesteps per core
NBLK = 8                     # score blocks (PSUM [32, 512] each, 16 l's)
LG = L_LOC // NBLK           # 16 l's per block
NCH = D // 128               # 16 contraction chunks

_CACHE: dict = {}
last_results = None          # BassKernelResults from the most recent run


def _split_drain_waits(nc):
    """Walrus rejects Drain instructions carrying many sync waits ("Too many
    sync wait commands").  Tile's kernel-tail drain waits on every live
    semaphore lane at once; split it into a chain of single-wait drains."""
    import concourse.mybir as mybir

    for bb in nc.main_func.blocks:
        idx = 0
        while idx < len(bb.instructions):
            inst = bb.instructions[idx]
            if (
                isinstance(inst, mybir.InstDrain)
                and inst.sync_info is not None
                and len(inst.sync_info.on_wait or []) > 1
            ):
                waits = list(inst.sync_info.on_wait)
                spill, keep = waits[:-1], waits[-1:]
                new_insts = []
                for j, w in enumerate(spill):
                    x = mybir.InstDrain(name=f"{inst.name}_w{j}", ins=[], outs=[])
                    x.engine = inst.engine
                    x.sync_info = mybir.SyncInfo(on_wait=[w], on_update=[])
                    x.debug = inst.debug
                    nc.register_instruction(x)
                    new_insts.append(x)
                inst.sync_info = mybir.SyncInfo(
                    on_wait=keep, on_update=list(inst.sync_info.on_update or [])
                )
                bb.instructions[idx:idx] = new_insts
                idx += len(new_insts)
            idx += 1


def build_program():
    """Build (once) the SPMD Bass program shared by all 8 cores."""
    if "nc" in _CACHE:
        return _CACHE["nc"]

    import concourse.bacc as bacc
    import concourse.mybir as mybir
    import concourse.tile as tile

    f32 = mybir.dt.float32
    bf16 = mybir.dt.bfloat16
    Alu = mybir.AluOpType
    Act = mybir.ActivationFunctionType

    nc = bacc.Bacc(
        "TRN2", target_bir_lowering=False, debug=False, num_devices=NCORES
    )

    enc = nc.dram_tensor(
        "enc", [NBLK * 128, NCH * 512], bf16, kind="ExternalInput"
    ).ap()
    # hbtt[p, 33c + j] = [hidden; b]^T[128c + p, j] — pre-tiled on the host so
    # the DMA is one contiguous run per partition.
    hbtt = nc.dram_tensor(
        "hbtt", [128, 16 * (B + 1)], bf16, kind="ExternalInput"
    ).ap()
    wfull = nc.dram_tensor("wfull", [D, D], bf16, kind="ExternalInput").ap()
    out = nc.dram_tensor("out", [L_LOC, B], f32, kind="ExternalOutput").ap()
    ident32 = nc.inline_tensor(np.eye(B, dtype=np.float32), "ident32").ap()
    ones32 = nc.inline_tensor(np.ones((B, 1), dtype=np.float32), "ones32").ap()
    # ohmat[:, 4*jj + m] = 1 if m == jj — row-select for the diag collapse
    oh_np = np.zeros((B, 16), dtype=np.float32)
    for jj in range(4):
        oh_np[:, 4 * jj + jj] = 1.0
    ohmat = nc.inline_tensor(np.ascontiguousarray(oh_np), "ohmat").ap()
    # mask[b', 32*lg + b] = 1 if b' == b — diagonal-extraction mask
    mask_np = np.tile(np.eye(B, dtype=np.float32), (1, LG))
    mask = nc.inline_tensor(np.ascontiguousarray(mask_np), "mask").ap()

    with tile.TileContext(nc) as tc:
        with (
            tc.tile_pool(name="pers", bufs=1) as pers,
            tc.tile_pool(name="encp", bufs=4) as encp,
            tc.tile_pool(name="wp", bufs=6) as wp,
            tc.tile_pool(name="mkp", bufs=2) as mkp,
            tc.tile_pool(name="psp", bufs=1, space="PSUM") as psp,
            tc.tile_pool(name="tpsp", bufs=2, space="PSUM") as tpsp,
        ):
            # ---------- phase 1: v = hidden @ W and c = hidden @ b on PE ----
            hbt_sb = pers.tile([128, 16 * (B + 1)], bf16)
            nc.sync.dma_start(hbt_sb[:, :], hbtt[:, :])
            mask_sb = pers.tile([B, LG * B], f32)
            nc.sync.dma_start(mask_sb[:, :], mask[:, :])
            ident_sb = pers.tile([B, B], f32)
            nc.sync.dma_start(ident_sb[:, :], ident32[:, :])
            ones_sb = pers.tile([B, 1], f32)
            nc.sync.dma_start(ones_sb[:, :], ones32[:, :])
            oh_sb = pers.tile([B, 16], f32)
            nc.sync.dma_start(oh_sb[:, :], ohmat[:, :])

            # PE warm-up: ~4 us of back-to-back matmuls flips the HAM clock
            # gate from 1.2 to 2.4 GHz before the W-chunk matmuls begin.
            warm_ps = psp.tile([128, 512], f32, tag="big")
            for i in range(12):
                nc.tensor.matmul(
                    warm_ps[:, :], hbt_sb[:, 0:128], hbt_sb[:, 0:512]
                )
            warm_junk = pers.tile([1, 1], f32)
            nc.vector.tensor_copy(warm_junk[:, :], warm_ps[0:1, 0:1])

            v_ps = psp.tile([B, D], f32, tag="big")
            c_ps = psp.tile([B, 1], f32, tag="sps1")
            last_w_dma = None
            for c in range(16):
                wc = wp.tile([128, D], bf16, tag="wc")
                last_w_dma = nc.sync.dma_start(
                    wc[:, :], wfull[128 * c : 128 * (c + 1), :]
                )
                lhs = hbt_sb[:, (B + 1) * c : (B + 1) * c + B]
                for n in range(4):
                    nc.tensor.matmul(
                        v_ps[:, 512 * n : 512 * (n + 1)],
                        lhs,
                        wc[:, 512 * n : 512 * (n + 1)],
                        start=(c == 0),
                        stop=(c == 15),
                    )
            for c in range(16):
                lhs = hbt_sb[:, (B + 1) * c : (B + 1) * c + B]
                rhs = hbt_sb[:, (B + 1) * c + B : (B + 1) * (c + 1)]
                nc.tensor.matmul(
                    c_ps[:, :], lhs, rhs, start=(c == 0), stop=(c == 15)
                )

            # ---------- phase 2: vT (PE transpose) + c replication ----------
            vsb = pers.tile([B, D], f32)
            nc.vector.tensor_copy(vsb[:, :], v_ps[:, :])
            vtr_ps = psp.tile([128, 512], f32, tag="sps0")
            for c in range(16):
                nc.tensor.transpose(
                    vtr_ps[:, B * c : B * (c + 1)],
                    vsb[:, 128 * c : 128 * (c + 1)],
                    ident_sb[:, :],
                )
            vT = pers.tile([128, NCH * B], bf16)  # [128, c, b'] = v[b', 128c+p]
            nc.vector.tensor_copy(vT[:, :], vtr_ps[:, :])

            # crep[jrow, 32*lg + b] = c[b]  (same for every jrow)
            c_sb = pers.tile([B, 1], f32)
            nc.vector.tensor_copy(c_sb[:, :], c_ps[:, :])
            cmask = pers.tile([B, LG * B], f32)
            nc.vector.tensor_scalar_mul(cmask[:, :], mask_sb[:, :], c_sb[:, 0:1])
            ones4 = pers.tile([B, 4], f32)
            for n in range(4):
                nc.vector.tensor_copy(ones4[:, n : n + 1], ones_sb[:, :])
            crep_ps = tpsp.tile([4, LG * B], f32, tag="T")
            nc.tensor.matmul(crep_ps[:, :], ones4[:, :], cmask[:, :])
            crep = pers.tile([4, LG * B], f32)
            nc.vector.tensor_copy(crep[:, :], crep_ps[:, :])
            # Preload the Exp/Ln ACT tables while the ACT engine is idle so
            # the epilogue doesn't eat the ~1.3 us ACT_TABLE_LOAD.
            tjunk = pers.tile([1, 2], f32)
            nc.scalar.activation(tjunk[:, 0:1], crep[0:1, 0:1], Act.Exp)
            nc.scalar.activation(tjunk[:, 1:2], crep[0:1, 0:1], Act.Ln)

            # ---------- phase 3: stream enc; multiply-reduce on the PE ------
            sps = [
                psp.tile([4, LG * B], f32, tag="sps0", name="sps0"),
                psp.tile([4, LG * B], f32, tag="sps1", name="sps1"),
            ]
            o_tiles = []
            for j in range(NBLK):
                et = encp.tile([128, NCH, 512], bf16, tag="et")
                enc_dma = nc.sync.dma_start(
                    et[:, :, :],
                    enc[128 * j : 128 * (j + 1), :].rearrange(
                        "p (c w) -> p c w", w=512
                    ),
                )
                # Keep the HWDGE FIFO ring W-first: vT must be ready early so
                # the PE can drain enc tiles as they land.
                tile.add_dep_helper(
                    enc_dma.ins,
                    last_w_dma.ins,
                    sync=False,
                    reason="enc stream after W (W-first DMA ordering)",
                )
                t_ps = tpsp.tile([B, LG * B], f32, tag="T")
                for c in range(NCH):
                    nc.tensor.matmul(
                        t_ps[:, :],
                        vT[:, B * c : B * (c + 1)],
                        et[:, c, :],
                        start=(c == 0),
                        stop=(c == NCH - 1),
                    )
                masked = mkp.tile([B, LG * B], f32, tag="mk")
                nc.vector.tensor_mul(masked[:, :], t_ps[:, :], mask_sb[:, :])
                jj = j % 4
                nc.tensor.matmul(
                    sps[j // 4][:, :],
                    oh_sb[:, 4 * jj : 4 * jj + 4],
                    masked[:, :],
                    start=(jj == 0),
                    stop=(jj == 3),
                )

                # ---------- epilogue per half: +c, log-softmax over b -------
                if j % 4 == 3:
                    h = j // 4
                    ssb = pers.tile([4, LG * B], f32, name=f"ssb{h}")
                    nc.vector.tensor_add(ssb[:, :], sps[h][:, :], crep[:, :])
                    mneg = pers.tile([4, LG], f32, name=f"mneg{h}")
                    nc.vector.tensor_reduce(
                        mneg[:, :],
                        ssb.rearrange("p (lg b) -> p lg b", b=B),
                        axis=mybir.AxisListType.X,
                        op=Alu.max,
                        negate=True,
                    )
                    sm = pers.tile([4, LG * B], f32, name=f"sm{h}")
                    for g in range(LG):
                        nc.vector.tensor_scalar_add(
                            sm[:, B * g : B * (g + 1)],
                            ssb[:, B * g : B * (g + 1)],
                            mneg[:, g : g + 1],
                        )
                    es = pers.tile([4, LG * B], f32, name=f"es{h}")
                    nc.scalar.activation(es[:, :], sm[:, :], Act.Exp)
                    s16 = pers.tile([4, LG], f32, name=f"s16{h}")
                    nc.vector.tensor_reduce(
                        s16[:, :],
                        es.rearrange("p (lg b) -> p lg b", b=B),
                        axis=mybir.AxisListType.X,
                        op=Alu.add,
                    )
                    ln16 = pers.tile([4, LG], f32, name=f"ln16{h}")
                    nc.scalar.activation(ln16[:, :], s16[:, :], Act.Ln)
                    o = pers.tile([4, LG * B], f32, name=f"o{h}")
                    for g in range(LG):
                        nc.vector.tensor_scalar_sub(
                            o[:, B * g : B * (g + 1)],
                            sm[:, B * g : B * (g + 1)],
                            ln16[:, g : g + 1],
                        )
                    o_tiles.append(o)
                    # out rows l = 64h + 16*jrow + lg ; one contiguous DMA
                    # per half on the scalar HWDGE queue (keeps the sync
                    # queue free for the enc stream).
                    out_h = out.rearrange("(h j lgb) b -> h j (lgb b)", h=2, j=4)
                    nc.scalar.dma_start(out_h[h, :, :], o[:, :])

    nc.compile()
    _split_drain_waits(nc)
    _CACHE["nc"] = nc
    return nc


def make_in_maps(hidden, encoder_outputs, W, b):
    import ml_dtypes

    bf16 = ml_dtypes.bfloat16
    hidden = np.asarray(hidden, dtype=np.float32)
    enc16 = np.asarray(encoder_outputs, dtype=np.float32).astype(bf16)
    W16 = np.ascontiguousarray(np.asarray(W, dtype=np.float32).astype(bf16))
    b_ = np.asarray(b, dtype=np.float32)
    hb = np.concatenate([hidden, b_[None, :]], axis=0)  # [33, D]
    # hbtt[p, 33c + j] = hb[j, 128c + p] — the SBUF tile layout, host-built
    hbtt16 = np.ascontiguousarray(
        hb.T.reshape(16, 128, B + 1)
        .transpose(1, 0, 2)
        .reshape(128, 16 * (B + 1))
        .astype(bf16)
    )
    in_maps = []
    for k in range(NCORES):
        # tile j, partition p, free (c, 32*lg+b) <- enc[l=16j+lg, b, 128c+p]
        ek = (
            enc16[k * L_LOC : (k + 1) * L_LOC]
            .reshape(NBLK, LG, B, NCH, 128)
            .transpose(0, 4, 3, 1, 2)
            .reshape(NBLK * 128, NCH * 512)
        )
        in_maps.append(
            {
                "enc": np.ascontiguousarray(ek),
                "hbtt": hbtt16,
                "wfull": W16,
            }
        )
    return in_maps


def kernel(hidden, encoder_outputs, W, b):
    """Full inputs in, full [1, 1, L, B] output out; runs on 8 NeuronCores."""
    global last_results
    from concourse.bass_utils import run_bass_kernel_spmd

    nc = build_program()
    in_maps = make_in_maps(hidden, encoder_outputs, W, b)
    res = run_bass_kernel_spmd(
        nc,
        in_maps,
        list(range(NCORES)),
        trace=bool(os.environ.get("KERNEL_TRACE")),
    )
    last_results = res
    chunks = [res.results[k]["out"] for k in range(NCORES)]
    full = np.concatenate(chunks, axis=0).reshape(1, 1, L, B)
    return full.astype(np.float32)
